# revision 75
# baseline (speedup 1.0000x reference)
"""Trainium2 Bass kernel for nn_Cluster_46574625358249 (vq_codebook).

Sharding: 4 fold-regions x 2 spatial-column-halves = 8 cores.

Host does all index prep AND the small 1x1 convs (it already needs the
feature/center matrices for the l2-norm scales): it ships, per core,
  fA [128,K]  fp16: rows 0-63  = hi(feat),  rows 64-127 = 0
  fB [128,K]  fp16: rows 0-63  = lo(feat),  rows 64-127 = hi(feat)
  cR [128,M]  fp16: rows 0-63  = hi(cnhat), rows 64-127 = lo(cnhat)
  vt [128,NKT*65] f32: per-column value vectors (+ kmask col 64)
  invx, beta128
where hi/lo is an exact float16 split (x ~= hi + lo to ~2^-22 rel), so the
device sim GEMM runs in TWO fp16 passes (1 cyc/row each) with fp32-grade
accuracy instead of one fp32 pass (4 cyc/row):
  az = fA^T @ cR  (hi*hi)   +   fB^T @ cR  (lo*hi + hi*lo)

Device per kt (K columns in 14 tiles of 128):
  az in three PSUM tile rings: azA0 [0:1024) x2 bufs (4 banks), azA1
    [1024:1604) x1 (2 banks), azB [1604:m_pad) x2 (2 banks); azA1's pool
    is declared first so its banks (freed earliest - its sign runs right
    after the sigmoid) host B2's accumulators at the phase seam
  rawmax  = max over m: three DVE reduces, combined + negated on Pool
  colval  = sigmoid(rawmax*invx + beta)  (ACT; invx ships negated)
  one-hot -> wA bf16 = Sign(az - rawmax) in {-1,0} on ALL m (ACT; host
    adds back the rowsum rs), order azA1/azB/azA0 to free rings fast
  rhsp    = bf16(vt * colval)            (Pool)
Then B2: agg[65, m] = sum_kt rhsp_kt @ wA_kt (bf16 GEMM, PSUM accumulate),
plus rs[65,1] = sum_kt rhsp_kt @ 1 for the host-side Sign correction.

Host combine: a = (half0 + half1) agg + rs; out = (a[:64] + vcT)/(a[64]+1),
tiny 64x64 projection, scatter back to point order.
"""

import numpy as np

FOLD_H = 2
FOLD_W = 2
SIZE_W = 1296.0
SIZE_H = 384.0
RH, RW = 32, 108          # folded region map H, W
HW = RH * RW              # 3456
K_HALF = HW // 2          # 1728
K_PAD = 1792              # 14*128
NKT = K_PAD // 128        # 14
M_PAD_DEFAULT = 2116      # >= max region count (2114) + 1 ghost
MZ = 1536                 # azA width (3 PSUM banks, double-buffered)
R = FOLD_H * FOLD_W
N_CORES = 8

_BUILT = {}
_LAST_IN_MAPS = None


def _build(m_pad):
    from concourse import bacc, mybir
    from concourse.tile import TileContext

    f32 = mybir.dt.float32
    f16 = mybir.dt.float16
    bf16 = mybir.dt.bfloat16
    m_b = m_pad - MZ
    # chunks must not cross 512-f32 PSUM bank boundaries
    aza_chunks = [(c, min(512, MZ - c)) for c in range(0, MZ, 512)]
    azb_chunks = [(c, min(512, m_b - c)) for c in range(0, m_b, 512)]
    m_chunks = [(c, min(512, m_pad - c)) for c in range(0, m_pad, 512)]
    cr_chunks = [(c, min(512, m_pad - c)) for c in range(0, m_pad, 512)]

    nc = bacc.Bacc(None, target_bir_lowering=False)
    fa_d = nc.dram_tensor("fA", [128, K_PAD], f16, kind="ExternalInput")
    fb_d = nc.dram_tensor("fB", [128, K_PAD], f16, kind="ExternalInput")
    cr_d = nc.dram_tensor("cR", [128, m_pad], f16, kind="ExternalInput")
    vt_d = nc.dram_tensor("vt", [128, NKT * 65], f32, kind="ExternalInput")
    invx_d = nc.dram_tensor("invx", [128, NKT], f32, kind="ExternalInput")
    beta_d = nc.dram_tensor("beta128", [128, 1], f32, kind="ExternalInput")
    agg_out = nc.dram_tensor("agg_out", [65, m_pad], f32, kind="ExternalOutput")
    rs_out = nc.dram_tensor("rs_out", [65, 1], f32, kind="ExternalOutput")

    Sig = mybir.ActivationFunctionType.Sigmoid
    Sgn = mybir.ActivationFunctionType.Sign
    Cpy = mybir.ActivationFunctionType.Copy
    X = mybir.AxisListType.X
    MAX = mybir.AluOpType.max
    MULT = mybir.AluOpType.mult
    IS_LE = mybir.AluOpType.is_le
    BYP = mybir.AluOpType.bypass

    with TileContext(nc) as tc:
        with tc.tile_pool(name="big", bufs=1) as big:
            # kt0's k-tile in separate head tiles (single DMA writer each)
            fAh = big.tile([128, 128], f16)
            fBh = big.tile([128, 128], f16)
            fA = big.tile([128, K_PAD], f16)
            fB = big.tile([128, K_PAD], f16)
            # cR as per-512-chunk tiles so kt0 isn't gated by the full DMA
            cRt = [big.tile([128, w], f16, name=f"cRt{ci}")
                   for ci, (_, w) in enumerate(cr_chunks)]
            crmap = [(t, 0) for t in cRt]
            vt = big.tile([128, NKT * 65], f32)
            invx = big.tile([128, NKT], f32)   # holds -1/|feat|
            beta = big.tile([128, 1], f32)
            # one-hot mask storage: ACT writes Sign(az-rmax) in {-1,0} for
            # ALL m (host adds back rowsum); Pool/GPSIMD cannot touch PSUM.
            wA = big.tile([128, NKT * m_pad], bf16)
            rhsp = big.tile([128, NKT * 65], bf16)
            ones1 = big.tile([128, 1], bf16)
            agg_sb = big.tile([65, m_pad], f32)
            rs_sb = big.tile([65, 1], f32)

            # DMAs: sync (fast HWDGE) carries kt0's critical inputs; gpsimd
            # (slow SWDGE, ~1.4us completion lag) carries later chunks; the
            # scalar queue takes invx first so the ACT table load (keyed off
            # the dummy sigmoid below) sequences behind it.
            nc.sync.dma_start(out=fAh[:], in_=fa_d[:, :128])
            nc.sync.dma_start(out=fBh[:], in_=fb_d[:, :128])
            for ci in (0, 1):
                off, w = cr_chunks[ci]
                nc.sync.dma_start(out=cRt[ci][:], in_=cr_d[:, off:off + w])
            for ci in (2, 3, 4):
                off, w = cr_chunks[ci]
                nc.gpsimd.dma_start(out=cRt[ci][:], in_=cr_d[:, off:off + w])
            for lo, hi in [(128, 576), (576, 1024), (1024, K_PAD)]:
                nc.gpsimd.dma_start(out=fA[:, lo:hi], in_=fa_d[:, lo:hi])
                nc.gpsimd.dma_start(out=fB[:, lo:hi], in_=fb_d[:, lo:hi])
            nc.scalar.dma_start(out=invx[:], in_=invx_d[:, :])
            nc.scalar.dma_start(out=beta[:], in_=beta_d[:, :])
            nc.scalar.dma_start(out=vt[:], in_=vt_d[:, :])
            nc.vector.memset(ones1[:], 1.0)

            # Pin the ACT function table (sigmoid set includes sign/copy)
            # before the loop so no mid-stream table swap occurs.
            dumm = big.tile([128, 1], f32)
            nc.scalar.activation(out=dumm[:], in_=invx[:, 0:1], func=Sig,
                                 bias=0.0, scale=1.0)

            # ---- B1: sim logits, column max, one-hot masks ----
            # az split into three PSUM tile rings: azA0 [0:1024) x2 bufs
            # (4 banks), azA1 [1024:1604) x1 (2 banks), azB [1604:m_pad)
            # x2 (2 banks). azA1 declared first: its banks free earliest
            # (its sign runs right after the sigmoid) for B2's accumulators.
            M1E = 1024 + 580
            MA1 = M1E - 1024
            mb2 = m_pad - M1E
            assert mb2 <= 512
            with tc.tile_pool(name="azA1", bufs=1, space="PSUM") as azA1p, \
                 tc.tile_pool(name="azB", bufs=2, space="PSUM") as azBp, \
                 tc.tile_pool(name="azA0", bufs=2, space="PSUM") as azA0p, \
                 tc.tile_pool(name="sm", bufs=4) as smp:
                for kt in range(NKT):
                    ksl = slice(kt * 128, (kt + 1) * 128)
                    lA = fAh[:] if kt == 0 else fA[:, ksl]
                    lB = fBh[:] if kt == 0 else fB[:, ksl]

                    def mm2(out_t, off_m, w, lA=lA, lB=lB):
                        ci = off_m // 512
                        t, base = crmap[ci]
                        co = base + off_m - cr_chunks[ci][0]
                        nc.tensor.matmul(out=out_t, lhsT=lA,
                                         rhs=t[:, co:co + w],
                                         start=True, stop=False)
                        nc.tensor.matmul(out=out_t, lhsT=lB,
                                         rhs=t[:, co:co + w],
                                         start=False, stop=True)

                    azA0 = azA0p.tile([128, 1024], f32, tag="azA0")
                    mm2(azA0[:, 0:512], 0, 512)
                    mm2(azA0[:, 512:1024], 512, 512)
                    azA1 = azA1p.tile([128, MA1], f32, tag="azA1")
                    mm2(azA1[:, 0:512], 1024, 512)
                    mm2(azA1[:, 512:MA1], 1536, MA1 - 512)
                    azB = azBp.tile([128, mb2], f32, tag="azB")
                    mm2(azB[:, 0:2048 - M1E], M1E, 2048 - M1E)
                    mm2(azB[:, 2048 - M1E:mb2], 2048, m_pad - 2048)

                    mxa0 = smp.tile([128, 1], f32, tag="mxa0")
                    nc.vector.reduce_max(out=mxa0[:], in_=azA0[:], axis=X)
                    mxa1 = smp.tile([128, 1], f32, tag="mxa1")
                    nc.vector.reduce_max(out=mxa1[:], in_=azA1[:], axis=X)
                    mxb = smp.tile([128, 1], f32, tag="mxb")
                    nc.vector.reduce_max(out=mxb[:], in_=azB[:], axis=X)
                    # Pool combines (SBUF-only): nrm = -max(all three)
                    mxa = smp.tile([128, 1], f32, tag="mxa")
                    nc.gpsimd.tensor_scalar(out=mxa[:], in0=mxa0[:],
                                            scalar1=mxa1[:], scalar2=None,
                                            op0=MAX)
                    nrm = smp.tile([128, 1], f32, tag="nrm")
                    nc.gpsimd.tensor_scalar(out=nrm[:], in0=mxb[:],
                                            scalar1=mxa[:], scalar2=-1.0,
                                            op0=MAX, op1=MULT)
                    colval = smp.tile([128, 1], f32, tag="cv")
                    nc.scalar.activation(out=colval[:], in_=nrm[:], func=Sig,
                                         bias=beta[:], scale=invx[:, kt:kt + 1])
                    # ACT: {-1,0} sign masks (host adds back rowsum over
                    # all m); azA1 first (the single-buffered ring)
                    wk = kt * m_pad
                    nc.scalar.activation(out=wA[:, wk + 1024:wk + M1E],
                                         in_=azA1[:], func=Sgn, bias=nrm[:])
                    nc.scalar.activation(out=wA[:, wk + M1E:wk + m_pad],
                                         in_=azB[:], func=Sgn, bias=nrm[:])
                    nc.scalar.activation(out=wA[:, wk:wk + 1024],
                                         in_=azA0[:], func=Sgn, bias=nrm[:])
                    nc.gpsimd.tensor_scalar(
                        out=rhsp[:, kt * 65:(kt + 1) * 65],
                        in0=vt[:, kt * 65:(kt + 1) * 65],
                        scalar1=colval[:], scalar2=None, op0=MULT)
            # ---- B2: aggregation GEMM (bf16), denominator in row 64 ----
            with tc.tile_pool(name="agg2", bufs=2, space="PSUM") as aggp, \
                 tc.tile_pool(name="rsp", bufs=1, space="PSUM") as rsp:
                rs = rsp.tile([65, 1], f32, tag="rs")
                for kt in range(NKT):
                    nc.tensor.matmul(out=rs[:],
                                     lhsT=rhsp[:, kt * 65:(kt + 1) * 65],
                                     rhs=ones1[:], start=(kt == 0),
                                     stop=(kt == NKT - 1))
                nc.vector.tensor_copy(out=rs_sb[:], in_=rs[:])
                nc.scalar.dma_start(out=rs_out[:, :], in_=rs_sb[:])
                # azB-range chunk first (its kt13 mask completes earliest);
                # tiny tail chunk last
                b2_order = [m_chunks[3], m_chunks[0], m_chunks[1],
                            m_chunks[2], m_chunks[4]]
                for off, w in b2_order:
                    ac = aggp.tile([65, 512], f32, tag="agg")
                    for kt in range(NKT):
                        wo = kt * m_pad + off
                        nc.tensor.matmul(
                            out=ac[:, :w],
                            lhsT=rhsp[:, kt * 65:(kt + 1) * 65],
                            rhs=wA[:, wo:wo + w],
                            start=(kt == 0), stop=(kt == NKT - 1))
                    nc.scalar.copy(out=agg_sb[:, off:off + w], in_=ac[:, :w])
                    nc.sync.dma_start(out=agg_out[:, off:off + w],
                                      in_=agg_sb[:, off:off + w])
    nc.compile()
    return nc


def _f32(x):
    return np.ascontiguousarray(np.asarray(x), dtype=np.float32)


def _region_indices(points):
    rh = np.float32(SIZE_H / FOLD_H)
    rw = np.float32(SIZE_W / FOLD_W)
    px, py = points[:, 0], points[:, 1]
    idxs = []
    for i in range(FOLD_H):
        for j in range(FOLD_W):
            m = (py > rh * i) & (py <= rh * (i + 1)) & \
                (px > rw * j) & (px <= rw * (j + 1))
            idxs.append(np.nonzero(m)[0])
    return idxs


def _bilinear_taps(pts):
    one = np.float32(1.0)
    gridx = pts[:, 0] / np.float32(SIZE_W - 1.0) * np.float32(2.0) - one
    gridy = pts[:, 1] / np.float32(SIZE_H - 1.0) * np.float32(2.0) - one
    gx = (gridx + one) * np.float32(RW * 0.5) - np.float32(0.5)
    gy = (gridy + one) * np.float32(RH * 0.5) - np.float32(0.5)
    x0 = np.floor(gx)
    y0 = np.floor(gy)
    wx = (gx - x0).astype(np.float32)
    wy = (gy - y0).astype(np.float32)
    x0i = np.clip(x0, 0, RW - 1).astype(np.int32)
    x1i = np.clip(x0 + 1.0, 0, RW - 1).astype(np.int32)
    y0i = np.clip(y0, 0, RH - 1).astype(np.int32)
    y1i = np.clip(y0 + 1.0, 0, RH - 1).astype(np.int32)
    taps = np.stack([y0i * RW + x0i, y0i * RW + x1i,
                     y1i * RW + x0i, y1i * RW + x1i], axis=1)
    w = np.stack([(one - wx) * (one - wy), wx * (one - wy),
                  (one - wx) * wy, wx * wy], axis=1).astype(np.float32)
    # Clamp-collapsed points (all 4 taps at one pixel, e.g. ghost slots and
    # border points): weight (1,0,0,0) makes those columns bit-identical to
    # the ghost column, so argmax ties are exact and deterministic.
    collapsed = (x0i == x1i) & (y0i == y1i)
    w[collapsed] = np.array([1.0, 0.0, 0.0, 0.0], np.float32)
    return taps, w


def _hilo(x):
    hi = x.astype(np.float16)
    lo = (x - hi.astype(np.float32)).astype(np.float16)
    return hi, lo


def kernel(points, x, W_f, b_f, W_v, b_v, W_proj, b_proj, sim_alpha, sim_beta):
    from concourse.bass_utils import run_bass_kernel_spmd

    points = _f32(points)[0]
    x = _f32(x)[0]
    W_f, b_f = _f32(W_f), _f32(b_f)
    W_v, b_v = _f32(W_v), _f32(b_v)
    W_proj, b_proj = _f32(W_proj), _f32(b_proj)
    alpha = _f32(sim_alpha).reshape(-1)[0]
    beta = _f32(sim_beta).reshape(-1)[0]
    N = points.shape[0]

    idxs = _region_indices(points)
    cnts = [len(ix) for ix in idxs]
    m_pad = M_PAD_DEFAULT
    need = max(cnts) + 1
    if need > m_pad:
        m_pad = ((need + 127) // 128) * 128

    Wfb = np.concatenate([W_f.T, b_f[None, :]], axis=0).astype(np.float32)
    Wvb = np.concatenate([W_v.T, b_v[None, :]], axis=0).astype(np.float32)
    beta128 = np.full((128, 1), beta, np.float32)

    in_maps = []
    vcts = []
    for r in range(R):
        i, j = divmod(r, FOLD_W)
        xr = x[:, i * RH:(i + 1) * RH, j * RW:(j + 1) * RW].reshape(64, HW)
        idx_r = idxs[r]
        cnt = len(idx_r)
        pts_r = np.zeros((m_pad, 2), np.float32)
        pts_r[:cnt] = points[idx_r]
        taps, w = _bilinear_taps(pts_r)
        g = xr[:, taps]                                    # [64, m_pad, 4]
        xg = np.einsum("cmt,mt->cm", g, w).astype(np.float32)
        xg1 = np.ascontiguousarray(
            np.concatenate([xg, np.ones((1, m_pad), np.float32)], axis=0))
        # centers + l2 scale (alpha folded in) -> scaled center features
        centers = (xg1.T @ Wfb).astype(np.float32)         # [m_pad, 64]
        nc2 = (centers * centers).sum(axis=1, dtype=np.float32)
        s = ((np.float32(1.0) / np.sqrt(nc2 + np.float32(1e-12))) * alpha
             ).astype(np.float32)
        cnhatT = np.ascontiguousarray((centers * s[:, None]).T)  # [64, m_pad]
        chi, clo = _hilo(cnhatT)
        cR = np.ascontiguousarray(np.concatenate([chi, clo], axis=0))
        # value centers (host side of the output combine)
        vcT = np.ascontiguousarray((xg1.T @ Wvb).T)        # [64, m_pad]
        vcts.append(vcT)
        # full-map features for sim columns + 1/|feat| scales
        xr1 = np.concatenate([xr, np.ones((1, HW), np.float32)], axis=0)
        featT = (xr1.T @ Wfb).astype(np.float32)           # [HW, 64]
        nfx = (featT * featT).sum(axis=1, dtype=np.float32)
        invx_full = (np.float32(1.0) / np.sqrt(nfx + np.float32(1e-12))
                     ).astype(np.float32)
        # per-column values (incl bias row + kmask col)
        vt_full = (xr1.T @ np.concatenate(
            [Wvb, np.zeros((65, 1), np.float32)], axis=1)).astype(np.float32)
        vt_full[:, 64] = 1.0                               # kmask for real k
        for h in range(2):
            fh = np.zeros((64, K_PAD), np.float32)
            fh[:, :K_HALF] = featT[h * K_HALF:(h + 1) * K_HALF].T
            fhi, flo = _hilo(fh)
            z = np.zeros_like(fhi)
            fA = np.ascontiguousarray(np.concatenate([fhi, z], axis=0))
            fB = np.ascontiguousarray(np.concatenate([flo, fhi], axis=0))
            vt_np = np.zeros((K_PAD, 65), np.float32)
            vt_np[:K_HALF] = vt_full[h * K_HALF:(h + 1) * K_HALF, :65]
            vt_in = np.ascontiguousarray(
                vt_np.reshape(NKT, 128, 65).transpose(1, 0, 2).reshape(
                    128, NKT * 65))
            # NEGATED: device computes sigmoid(invx * (-rawmax) + beta)
            iv = np.full((K_PAD,), -1e6, np.float32)
            iv[:K_HALF] = -invx_full[h * K_HALF:(h + 1) * K_HALF]
            invx = np.ascontiguousarray(iv.reshape(NKT, 128).T)   # [128, NKT]
            in_maps.append({
                "fA": fA, "fB": fB, "cR": cR, "vt": vt_in,
                "invx": invx, "beta128": beta128,
            })

    global _LAST_IN_MAPS
    _LAST_IN_MAPS = in_maps
    if m_pad not in _BUILT:
        _BUILT[m_pad] = _build(m_pad)
    res = run_bass_kernel_spmd(_BUILT[m_pad], in_maps,
                               core_ids=list(range(N_CORES)))
    results = res.results

    out = np.zeros((64, N), np.float32)
    for r in range(R):
        a = results[2 * r]["agg_out"] + results[2 * r + 1]["agg_out"]
        rs = results[2 * r]["rs_out"] + results[2 * r + 1]["rs_out"]  # [65,1]
        a += rs
        vcT = vcts[r]
        idx_r = idxs[r]
        cnt = len(idx_r)
        ort = (a[:64, :cnt] + vcT[:, :cnt]) / \
            (a[64, :cnt] + np.float32(1.0))[None, :]
        proj = W_proj @ ort + b_proj[:, None]
        mask = np.any(ort != 0.0, axis=0)
        out[:, idx_r] = proj * mask[None, :]
    return out[None, :, None, :]


# revision 76
# speedup vs baseline: 1.0017x; 1.0017x over previous
"""Trainium2 Bass kernel for nn_Cluster_46574625358249 (vq_codebook).

Sharding: 4 fold-regions x 2 spatial-column-halves = 8 cores.

Host does all index prep AND the small 1x1 convs (it already needs the
feature/center matrices for the l2-norm scales): it ships, per core,
  fA [128,K]  fp16: rows 0-63  = hi(feat),  rows 64-127 = 0
  fB [128,K]  fp16: rows 0-63  = lo(feat),  rows 64-127 = hi(feat)
  cR [128,M]  fp16: rows 0-63  = hi(cnhat), rows 64-127 = lo(cnhat)
  vt [128,NKT*65] f32: per-column value vectors (+ kmask col 64)
  invx, beta128
where hi/lo is an exact float16 split (x ~= hi + lo to ~2^-22 rel), so the
device sim GEMM runs in TWO fp16 passes (1 cyc/row each) with fp32-grade
accuracy instead of one fp32 pass (4 cyc/row):
  az = fA^T @ cR  (hi*hi)   +   fB^T @ cR  (lo*hi + hi*lo)

Device per kt (K columns in 14 tiles of 128):
  az in three PSUM tile rings: azA0 [0:1024) x2 bufs (4 banks), azA1
    [1024:1604) x1 (2 banks), azB [1604:m_pad) x2 (2 banks); azA1's pool
    is declared first so its banks (freed earliest - its sign runs right
    after the sigmoid) host B2's accumulators at the phase seam
  rawmax  = max over m: three DVE reduces, combined + negated on Pool
  colval  = sigmoid(rawmax*invx + beta)  (ACT; invx ships negated)
  one-hot -> wA bf16 = Sign(az - rawmax) in {-1,0} on ALL m (ACT; host
    adds back the rowsum rs), order azA1/azB/azA0 to free rings fast
  rhsp    = bf16(vt * colval)            (Pool)
Then B2: agg[65, m] = sum_kt rhsp_kt @ wA_kt (bf16 GEMM, PSUM accumulate),
plus rs[65,1] = sum_kt rhsp_kt @ 1 for the host-side Sign correction.

Host combine: a = (half0 + half1) agg + rs; out = (a[:64] + vcT)/(a[64]+1),
tiny 64x64 projection, scatter back to point order.
"""

import numpy as np

FOLD_H = 2
FOLD_W = 2
SIZE_W = 1296.0
SIZE_H = 384.0
RH, RW = 32, 108          # folded region map H, W
HW = RH * RW              # 3456
K_HALF = HW // 2          # 1728
K_PAD = 1792              # 14*128
NKT = K_PAD // 128        # 14
M_PAD_DEFAULT = 2116      # >= max region count (2114) + 1 ghost
MZ = 1536                 # azA width (3 PSUM banks, double-buffered)
R = FOLD_H * FOLD_W
N_CORES = 8

_BUILT = {}
_LAST_IN_MAPS = None


def _build(m_pad):
    from concourse import bacc, mybir
    from concourse.tile import TileContext

    f32 = mybir.dt.float32
    f16 = mybir.dt.float16
    bf16 = mybir.dt.bfloat16
    m_b = m_pad - MZ
    # chunks must not cross 512-f32 PSUM bank boundaries
    aza_chunks = [(c, min(512, MZ - c)) for c in range(0, MZ, 512)]
    azb_chunks = [(c, min(512, m_b - c)) for c in range(0, m_b, 512)]
    m_chunks = [(c, min(512, m_pad - c)) for c in range(0, m_pad, 512)]
    cr_chunks = [(c, min(512, m_pad - c)) for c in range(0, m_pad, 512)]

    nc = bacc.Bacc(None, target_bir_lowering=False)
    fa_d = nc.dram_tensor("fA", [128, K_PAD], f16, kind="ExternalInput")
    fb_d = nc.dram_tensor("fB", [128, K_PAD], f16, kind="ExternalInput")
    cr_d = nc.dram_tensor("cR", [128, m_pad], f16, kind="ExternalInput")
    vt_d = nc.dram_tensor("vt", [128, NKT * 65], f32, kind="ExternalInput")
    invx_d = nc.dram_tensor("invx", [128, NKT], f32, kind="ExternalInput")
    beta_d = nc.dram_tensor("beta128", [128, 1], f32, kind="ExternalInput")
    agg_out = nc.dram_tensor("agg_out", [65, m_pad], f32, kind="ExternalOutput")
    rs_out = nc.dram_tensor("rs_out", [65, 1], f32, kind="ExternalOutput")

    Sig = mybir.ActivationFunctionType.Sigmoid
    Sgn = mybir.ActivationFunctionType.Sign
    Cpy = mybir.ActivationFunctionType.Copy
    X = mybir.AxisListType.X
    MAX = mybir.AluOpType.max
    MULT = mybir.AluOpType.mult
    IS_LE = mybir.AluOpType.is_le
    BYP = mybir.AluOpType.bypass

    with TileContext(nc) as tc:
        with tc.tile_pool(name="big", bufs=1) as big:
            # kt0's k-tile in separate head tiles (single DMA writer each)
            fAh = big.tile([128, 128], f16)
            fBh = big.tile([128, 128], f16)
            fA = big.tile([128, K_PAD], f16)
            fB = big.tile([128, K_PAD], f16)
            # cR as per-512-chunk tiles so kt0 isn't gated by the full DMA
            cRt = [big.tile([128, w], f16, name=f"cRt{ci}")
                   for ci, (_, w) in enumerate(cr_chunks)]
            crmap = [(t, 0) for t in cRt]
            vt = big.tile([128, NKT * 65], f32)
            invx = big.tile([128, NKT], f32)   # holds -1/|feat|
            beta = big.tile([128, 1], f32)
            # one-hot mask storage: ACT writes Sign(az-rmax) in {-1,0} for
            # ALL m (host adds back rowsum); Pool/GPSIMD cannot touch PSUM.
            wA = big.tile([128, NKT * m_pad], bf16)
            rhsp = big.tile([128, NKT * 65], bf16)
            ones1 = big.tile([128, 1], bf16)
            agg_sb = big.tile([65, m_pad], f32)
            rs_sb = big.tile([65, 1], f32)

            # DMAs: sync (fast HWDGE) carries kt0's critical inputs; gpsimd
            # (slow SWDGE, ~1.4us completion lag) carries later chunks; the
            # scalar queue takes invx first so the ACT table load (keyed off
            # the dummy sigmoid below) sequences behind it.
            nc.sync.dma_start(out=fAh[:], in_=fa_d[:, :128])
            nc.sync.dma_start(out=fBh[:], in_=fb_d[:, :128])
            for ci in (0, 1):
                off, w = cr_chunks[ci]
                nc.sync.dma_start(out=cRt[ci][:], in_=cr_d[:, off:off + w])
            for ci in (2, 3, 4):
                off, w = cr_chunks[ci]
                nc.gpsimd.dma_start(out=cRt[ci][:], in_=cr_d[:, off:off + w])
            for lo, hi in [(128, 576), (576, 1024), (1024, K_PAD)]:
                nc.gpsimd.dma_start(out=fA[:, lo:hi], in_=fa_d[:, lo:hi])
                nc.gpsimd.dma_start(out=fB[:, lo:hi], in_=fb_d[:, lo:hi])
            nc.scalar.dma_start(out=invx[:], in_=invx_d[:, :])
            nc.scalar.dma_start(out=beta[:], in_=beta_d[:, :])
            nc.scalar.dma_start(out=vt[:], in_=vt_d[:, :])
            nc.vector.memset(ones1[:], 1.0)

            # Pin the ACT function table (sigmoid set includes sign/copy)
            # before the loop so no mid-stream table swap occurs.
            dumm = big.tile([128, 1], f32)
            nc.scalar.activation(out=dumm[:], in_=invx[:, 0:1], func=Sig,
                                 bias=0.0, scale=1.0)

            # ---- B1: sim logits, column max, one-hot masks ----
            # az split into three PSUM tile rings: azA0 [0:1024) x2 bufs
            # (4 banks), azA1 [1024:1604) x1 (2 banks), azB [1604:m_pad)
            # x2 (2 banks). azA1 declared first: its banks free earliest
            # (its sign runs right after the sigmoid) for B2's accumulators.
            M1E = 1024 + 580
            MA1 = M1E - 1024
            mb2 = m_pad - M1E
            assert mb2 <= 512
            with tc.tile_pool(name="azA1", bufs=1, space="PSUM") as azA1p, \
                 tc.tile_pool(name="azB", bufs=2, space="PSUM") as azBp, \
                 tc.tile_pool(name="azA0", bufs=2, space="PSUM") as azA0p, \
                 tc.tile_pool(name="sm", bufs=4) as smp:
                for kt in range(NKT):
                    ksl = slice(kt * 128, (kt + 1) * 128)
                    lA = fAh[:] if kt == 0 else fA[:, ksl]
                    lB = fBh[:] if kt == 0 else fB[:, ksl]

                    def mm2(out_t, off_m, w, lA=lA, lB=lB):
                        ci = off_m // 512
                        t, base = crmap[ci]
                        co = base + off_m - cr_chunks[ci][0]
                        nc.tensor.matmul(out=out_t, lhsT=lA,
                                         rhs=t[:, co:co + w],
                                         start=True, stop=False)
                        nc.tensor.matmul(out=out_t, lhsT=lB,
                                         rhs=t[:, co:co + w],
                                         start=False, stop=True)

                    azA0 = azA0p.tile([128, 1024], f32, tag="azA0")
                    mm2(azA0[:, 0:512], 0, 512)
                    mm2(azA0[:, 512:1024], 512, 512)
                    azA1 = azA1p.tile([128, MA1], f32, tag="azA1")
                    mm2(azA1[:, 0:512], 1024, 512)
                    mm2(azA1[:, 512:MA1], 1536, MA1 - 512)
                    azB = azBp.tile([128, mb2], f32, tag="azB")
                    mm2(azB[:, 0:2048 - M1E], M1E, 2048 - M1E)
                    mm2(azB[:, 2048 - M1E:mb2], 2048, m_pad - 2048)

                    mxa0 = smp.tile([128, 1], f32, tag="mxa0")
                    nc.vector.reduce_max(out=mxa0[:], in_=azA0[:], axis=X)
                    mxa1 = smp.tile([128, 1], f32, tag="mxa1")
                    nc.vector.reduce_max(out=mxa1[:], in_=azA1[:], axis=X)
                    mxb = smp.tile([128, 1], f32, tag="mxb")
                    nc.vector.reduce_max(out=mxb[:], in_=azB[:], axis=X)
                    # Pool combines (SBUF-only): nrm = -max(all three)
                    mxa = smp.tile([128, 1], f32, tag="mxa")
                    nc.gpsimd.tensor_scalar(out=mxa[:], in0=mxa0[:],
                                            scalar1=mxa1[:], scalar2=None,
                                            op0=MAX)
                    nrm = smp.tile([128, 1], f32, tag="nrm")
                    nc.gpsimd.tensor_scalar(out=nrm[:], in0=mxb[:],
                                            scalar1=mxa[:], scalar2=-1.0,
                                            op0=MAX, op1=MULT)
                    # ACT: {-1,0} sign masks (host adds back rowsum over
                    # all m); azA1 first (the single-buffered ring), the
                    # sigmoid last (colval is only needed by B2's rhsp)
                    wk = kt * m_pad
                    nc.scalar.activation(out=wA[:, wk + 1024:wk + M1E],
                                         in_=azA1[:], func=Sgn, bias=nrm[:])
                    nc.scalar.activation(out=wA[:, wk + M1E:wk + m_pad],
                                         in_=azB[:], func=Sgn, bias=nrm[:])
                    nc.scalar.activation(out=wA[:, wk:wk + 1024],
                                         in_=azA0[:], func=Sgn, bias=nrm[:])
                    colval = smp.tile([128, 1], f32, tag="cv")
                    nc.scalar.activation(out=colval[:], in_=nrm[:], func=Sig,
                                         bias=beta[:], scale=invx[:, kt:kt + 1])
                    nc.gpsimd.tensor_scalar(
                        out=rhsp[:, kt * 65:(kt + 1) * 65],
                        in0=vt[:, kt * 65:(kt + 1) * 65],
                        scalar1=colval[:], scalar2=None, op0=MULT)
            # ---- B2: aggregation GEMM (bf16), denominator in row 64 ----
            with tc.tile_pool(name="agg2", bufs=2, space="PSUM") as aggp, \
                 tc.tile_pool(name="rsp", bufs=1, space="PSUM") as rsp:
                rs = rsp.tile([65, 1], f32, tag="rs")
                for kt in range(NKT):
                    nc.tensor.matmul(out=rs[:],
                                     lhsT=rhsp[:, kt * 65:(kt + 1) * 65],
                                     rhs=ones1[:], start=(kt == 0),
                                     stop=(kt == NKT - 1))
                nc.vector.tensor_copy(out=rs_sb[:], in_=rs[:])
                nc.scalar.dma_start(out=rs_out[:, :], in_=rs_sb[:])
                # azB-range chunk first (its kt13 mask completes earliest);
                # tiny tail chunk last
                b2_order = [m_chunks[3], m_chunks[0], m_chunks[1],
                            m_chunks[2], m_chunks[4]]
                for off, w in b2_order:
                    ac = aggp.tile([65, 512], f32, tag="agg")
                    for kt in range(NKT):
                        wo = kt * m_pad + off
                        nc.tensor.matmul(
                            out=ac[:, :w],
                            lhsT=rhsp[:, kt * 65:(kt + 1) * 65],
                            rhs=wA[:, wo:wo + w],
                            start=(kt == 0), stop=(kt == NKT - 1))
                    nc.scalar.copy(out=agg_sb[:, off:off + w], in_=ac[:, :w])
                    nc.sync.dma_start(out=agg_out[:, off:off + w],
                                      in_=agg_sb[:, off:off + w])
    nc.compile()
    return nc


def _f32(x):
    return np.ascontiguousarray(np.asarray(x), dtype=np.float32)


def _region_indices(points):
    rh = np.float32(SIZE_H / FOLD_H)
    rw = np.float32(SIZE_W / FOLD_W)
    px, py = points[:, 0], points[:, 1]
    idxs = []
    for i in range(FOLD_H):
        for j in range(FOLD_W):
            m = (py > rh * i) & (py <= rh * (i + 1)) & \
                (px > rw * j) & (px <= rw * (j + 1))
            idxs.append(np.nonzero(m)[0])
    return idxs


def _bilinear_taps(pts):
    one = np.float32(1.0)
    gridx = pts[:, 0] / np.float32(SIZE_W - 1.0) * np.float32(2.0) - one
    gridy = pts[:, 1] / np.float32(SIZE_H - 1.0) * np.float32(2.0) - one
    gx = (gridx + one) * np.float32(RW * 0.5) - np.float32(0.5)
    gy = (gridy + one) * np.float32(RH * 0.5) - np.float32(0.5)
    x0 = np.floor(gx)
    y0 = np.floor(gy)
    wx = (gx - x0).astype(np.float32)
    wy = (gy - y0).astype(np.float32)
    x0i = np.clip(x0, 0, RW - 1).astype(np.int32)
    x1i = np.clip(x0 + 1.0, 0, RW - 1).astype(np.int32)
    y0i = np.clip(y0, 0, RH - 1).astype(np.int32)
    y1i = np.clip(y0 + 1.0, 0, RH - 1).astype(np.int32)
    taps = np.stack([y0i * RW + x0i, y0i * RW + x1i,
                     y1i * RW + x0i, y1i * RW + x1i], axis=1)
    w = np.stack([(one - wx) * (one - wy), wx * (one - wy),
                  (one - wx) * wy, wx * wy], axis=1).astype(np.float32)
    # Clamp-collapsed points (all 4 taps at one pixel, e.g. ghost slots and
    # border points): weight (1,0,0,0) makes those columns bit-identical to
    # the ghost column, so argmax ties are exact and deterministic.
    collapsed = (x0i == x1i) & (y0i == y1i)
    w[collapsed] = np.array([1.0, 0.0, 0.0, 0.0], np.float32)
    return taps, w


def _hilo(x):
    hi = x.astype(np.float16)
    lo = (x - hi.astype(np.float32)).astype(np.float16)
    return hi, lo


def kernel(points, x, W_f, b_f, W_v, b_v, W_proj, b_proj, sim_alpha, sim_beta):
    from concourse.bass_utils import run_bass_kernel_spmd

    points = _f32(points)[0]
    x = _f32(x)[0]
    W_f, b_f = _f32(W_f), _f32(b_f)
    W_v, b_v = _f32(W_v), _f32(b_v)
    W_proj, b_proj = _f32(W_proj), _f32(b_proj)
    alpha = _f32(sim_alpha).reshape(-1)[0]
    beta = _f32(sim_beta).reshape(-1)[0]
    N = points.shape[0]

    idxs = _region_indices(points)
    cnts = [len(ix) for ix in idxs]
    m_pad = M_PAD_DEFAULT
    need = max(cnts) + 1
    if need > m_pad:
        m_pad = ((need + 127) // 128) * 128

    Wfb = np.concatenate([W_f.T, b_f[None, :]], axis=0).astype(np.float32)
    Wvb = np.concatenate([W_v.T, b_v[None, :]], axis=0).astype(np.float32)
    beta128 = np.full((128, 1), beta, np.float32)

    in_maps = []
    vcts = []
    for r in range(R):
        i, j = divmod(r, FOLD_W)
        xr = x[:, i * RH:(i + 1) * RH, j * RW:(j + 1) * RW].reshape(64, HW)
        idx_r = idxs[r]
        cnt = len(idx_r)
        pts_r = np.zeros((m_pad, 2), np.float32)
        pts_r[:cnt] = points[idx_r]
        taps, w = _bilinear_taps(pts_r)
        g = xr[:, taps]                                    # [64, m_pad, 4]
        xg = np.einsum("cmt,mt->cm", g, w).astype(np.float32)
        xg1 = np.ascontiguousarray(
            np.concatenate([xg, np.ones((1, m_pad), np.float32)], axis=0))
        # centers + l2 scale (alpha folded in) -> scaled center features
        centers = (xg1.T @ Wfb).astype(np.float32)         # [m_pad, 64]
        nc2 = (centers * centers).sum(axis=1, dtype=np.float32)
        s = ((np.float32(1.0) / np.sqrt(nc2 + np.float32(1e-12))) * alpha
             ).astype(np.float32)
        cnhatT = np.ascontiguousarray((centers * s[:, None]).T)  # [64, m_pad]
        chi, clo = _hilo(cnhatT)
        cR = np.ascontiguousarray(np.concatenate([chi, clo], axis=0))
        # value centers (host side of the output combine)
        vcT = np.ascontiguousarray((xg1.T @ Wvb).T)        # [64, m_pad]
        vcts.append(vcT)
        # full-map features for sim columns + 1/|feat| scales
        xr1 = np.concatenate([xr, np.ones((1, HW), np.float32)], axis=0)
        featT = (xr1.T @ Wfb).astype(np.float32)           # [HW, 64]
        nfx = (featT * featT).sum(axis=1, dtype=np.float32)
        invx_full = (np.float32(1.0) / np.sqrt(nfx + np.float32(1e-12))
                     ).astype(np.float32)
        # per-column values (incl bias row + kmask col)
        vt_full = (xr1.T @ np.concatenate(
            [Wvb, np.zeros((65, 1), np.float32)], axis=1)).astype(np.float32)
        vt_full[:, 64] = 1.0                               # kmask for real k
        for h in range(2):
            fh = np.zeros((64, K_PAD), np.float32)
            fh[:, :K_HALF] = featT[h * K_HALF:(h + 1) * K_HALF].T
            fhi, flo = _hilo(fh)
            z = np.zeros_like(fhi)
            fA = np.ascontiguousarray(np.concatenate([fhi, z], axis=0))
            fB = np.ascontiguousarray(np.concatenate([flo, fhi], axis=0))
            vt_np = np.zeros((K_PAD, 65), np.float32)
            vt_np[:K_HALF] = vt_full[h * K_HALF:(h + 1) * K_HALF, :65]
            vt_in = np.ascontiguousarray(
                vt_np.reshape(NKT, 128, 65).transpose(1, 0, 2).reshape(
                    128, NKT * 65))
            # NEGATED: device computes sigmoid(invx * (-rawmax) + beta)
            iv = np.full((K_PAD,), -1e6, np.float32)
            iv[:K_HALF] = -invx_full[h * K_HALF:(h + 1) * K_HALF]
            invx = np.ascontiguousarray(iv.reshape(NKT, 128).T)   # [128, NKT]
            in_maps.append({
                "fA": fA, "fB": fB, "cR": cR, "vt": vt_in,
                "invx": invx, "beta128": beta128,
            })

    global _LAST_IN_MAPS
    _LAST_IN_MAPS = in_maps
    if m_pad not in _BUILT:
        _BUILT[m_pad] = _build(m_pad)
    res = run_bass_kernel_spmd(_BUILT[m_pad], in_maps,
                               core_ids=list(range(N_CORES)))
    results = res.results

    out = np.zeros((64, N), np.float32)
    for r in range(R):
        a = results[2 * r]["agg_out"] + results[2 * r + 1]["agg_out"]
        rs = results[2 * r]["rs_out"] + results[2 * r + 1]["rs_out"]  # [65,1]
        a += rs
        vcT = vcts[r]
        idx_r = idxs[r]
        cnt = len(idx_r)
        ort = (a[:64, :cnt] + vcT[:, :cnt]) / \
            (a[64, :cnt] + np.float32(1.0))[None, :]
        proj = W_proj @ ort + b_proj[:, None]
        mask = np.any(ort != 0.0, axis=0)
        out[:, idx_r] = proj * mask[None, :]
    return out[None, :, None, :]
